# revision 3
# baseline (speedup 1.0000x reference)
"""DiffLinearAttentionWeights Trainium2 kernel.

Math (per b, h):
    aw_i = (q @ Wq_i) @ (k @ Wk_i)^T  = q @ M_i @ k^T,   M_i = Wq_i @ Wk_i^T
    masked with tril(k=1), row-normalized; out = aw_1/den_1 - lam * aw_2/den_2.

On-device factorization (per bh):
  * U^T_stack = Ms^T @ qT   (Ms = [M1|M2], fp32 matmul, K=64).
  * den_i[t] = U_i[t] . C[t+1], C = prefix-sum of k rows (one DVE scan).
    W = Ustack * shift(C2); den = ones^T @ W (fp32 matmul).
    The den pipeline stays fp32: den cancellations amplify rounding, and
    f32r rounds to ~13 mantissa bits on HW.
  * V = Ustack * [1/den1 ; -lam/den2] (reciprocal + the lambda combination
    folded via the host-built `on` matrix).
  * out_tile = V^T @ [kT;kT] in float32r (1 cyc/row vs 4 for fp32): the
    aw-value path tolerates ~1e-4 relative rounding.
  * Only tiles under the causal profile are computed/written; PJRT output
    buffers are donated zero-filled arrays, so skipped regions stay zero.

Schedule: software pipeline out(j) / front(j+2) / back(j+1); kT is
written to both kT2 halves directly from the transpose PSUM tiles (one
lane-aligned copy + one partition-shifted ACT copy) so no duplication
DMA is needed; per-tile drains fold the causal mask and the full-keep
left columns into a single DVE mask-multiply via an extended mask
constant; 4 PSUM banks for the output matmuls keep the out-DMA stream
saturated. CoreSim cost model: ~76.9 us per core (DMA-bound: 52.5 us
output writes + 11.6 us input reads at ~360 GB/s).

Sharding: BH = 64 (b,h) pairs, 8 per core, SPMD on 8 NeuronCores.
"""

import math
import sys

sys.path.insert(0, "/opt/trn_rl_repo")

import numpy as np

B, H, T, D = 4, 16, 1024, 64
NCORES = 8
BH = B * H
JPC = BH // NCORES          # bh pairs per core
NT = T // 128               # t-chunks of 128 rows
DEPTH = 12
LAMBDA_INIT = 0.8 - 0.6 * math.exp(-0.3 * DEPTH)

# live width of output row-block i: causal tril(k=1) keeps cols 0..128*(i+1)+1
def _live_width(i):
    return min(128 * (i + 1) + 1, T)


_BUILD_CACHE = {}


def _build_module(n_bh=JPC, repeat=1):
    """Trace + compile the per-core Bass module (cached).

    repeat>1 wraps the whole body in a hardware For_i loop that re-executes
    the identical computation `repeat` times per dispatch (same inputs, same
    outputs — idempotent). Used only by test.py to time the kernel free of
    per-dispatch host/RPC overhead; kernel() always uses repeat=1.
    """
    if (n_bh, repeat) in _BUILD_CACHE:
        return _BUILD_CACHE[(n_bh, repeat)]

    import concourse.bass as bass
    import concourse.mybir as mybir
    import concourse.bacc as bacc
    import concourse.tile as tile
    from concourse import masks

    fp32 = mybir.dt.float32
    f32r = mybir.dt.float32r
    P = 128

    nc = bacc.Bacc("TRN2", target_bir_lowering=False, debug=False,
                   enable_asserts=False)

    q_d = nc.dram_tensor("q", [n_bh, T, D], fp32, kind="ExternalInput")
    k_d = nc.dram_tensor("k", [n_bh, T, D], fp32, kind="ExternalInput")
    # ms[j] = [M1 | M2]  (64 x 128)
    ms_d = nc.dram_tensor("ms", [n_bh, D, 2 * D], fp32, kind="ExternalInput")
    # on: cols 0..63 = 1 on rows 0..63; cols 64..127 = -1/lam on rows 64..127
    on_d = nc.dram_tensor("on", [P, P], fp32, kind="ExternalInput")
    out_d = nc.dram_tensor("out", [n_bh, T, T], fp32, kind="ExternalOutput")

    with tile.TileContext(nc) as tc:
        with tc.tile_pool(name="const", bufs=1) as cpool, \
             tc.tile_pool(name="nat", bufs=2) as natp, \
             tc.tile_pool(name="big", bufs=2) as big, \
             tc.tile_pool(name="outp", bufs=2) as outp, \
             tc.tile_pool(name="trp", bufs=2, space=bass.MemorySpace.PSUM) as trp, \
             tc.tile_pool(name="usp", bufs=1, space=bass.MemorySpace.PSUM) as usp, \
             tc.tile_pool(name="denp", bufs=1, space=bass.MemorySpace.PSUM) as denp, \
             tc.tile_pool(name="owp", bufs=4, space=bass.MemorySpace.PSUM) as owp:

            # ---- constants ----
            ident = cpool.tile([P, P], fp32)
            masks.make_identity(nc, ident[:])
            # extended mask: 512 all-ones cols then the tril(k=1) strip
            # pattern, so one mask-mul handles left cols + diagonal strip
            mext = cpool.tile([P, 648], fp32)
            nc.gpsimd.memset(mext[:], 1.0)
            nc.gpsimd.affine_select(
                out=mext[:], in_=mext[:], compare_op=mybir.AluOpType.is_ge,
                fill=0.0, base=513, pattern=[[-1, 648]], channel_multiplier=1)
            on_sb = cpool.tile([P, P], fp32)
            ms_sb = cpool.tile([D, n_bh, 2 * D], fp32)
            warm = cpool.tile([P, 1], fp32)

            def act_warmup():
                # trigger LoadActFuncSet while the first input DMA runs
                nc.scalar.copy(warm[:], ident[:, 0:1])

            def load_consts():
                nc.gpsimd.dma_start(ms_sb[:], ms_d.rearrange("j d m -> d j m"))
                nc.gpsimd.dma_start(on_sb[:], on_d[:])

            st = {}      # per-j live tiles
            pair_st = {}

            def load_pair(pr, split=False):
                j0 = 2 * pr
                qnat = natp.tile([P, 2, NT, D], fp32, tag="qnat")
                knat = natp.tile([P, 2, NT, D], fp32, tag="knat")
                if split:
                    for jj in range(2):
                        nc.sync.dma_start(
                            qnat[:, jj], q_d[j0 + jj].rearrange(
                                "(p e) d -> p e d", p=P))
                        nc.sync.dma_start(
                            knat[:, jj], k_d[j0 + jj].rearrange(
                                "(p e) d -> p e d", p=P))
                else:
                    nc.gpsimd.dma_start(
                        knat[:],
                        k_d[j0:j0 + 2].rearrange("jj (p e) d -> p jj e d", p=P))
                    nc.gpsimd.dma_start(
                        qnat[:],
                        q_d[j0:j0 + 2].rearrange("jj (p e) d -> p jj e d", p=P))
                pair_st[pr] = (qnat, knat)

            def front(j):
                pr, jj = divmod(j, 2)
                if jj == 0 and pr not in pair_st:
                    load_pair(pr)
                qnat, knat = pair_st[pr]

                qT = big.tile([D, T], fp32, tag="qT")
                kT2 = big.tile([P, T], fp32, tag="kT2")
                # q first: the qT -> U -> ust -> W chain is longer than the
                # kT -> scan -> W chain. kT is written twice: lane-aligned to
                # kT2[0:64] (DVE) and partition-shifted to kT2[64:128] (ACT).
                for src, dsttile in ((qnat, qT), (knat, kT2)):
                    for g in range(2):
                        tp = trp.tile([D, 512], fp32, tag="tr")
                        for ee in range(4):
                            e = 4 * g + ee
                            nc.tensor.transpose(tp[:, 128 * ee:128 * (ee + 1)],
                                                src[:, jj, e, :], ident[:])
                        tpr = tp[:].rearrange("p (ee c) -> p ee c", ee=4)
                        dst = dsttile[0:D, :].rearrange(
                            "p (c e) -> p e c", e=8)[:, 4 * g:4 * (g + 1), :]
                        if src is knat:
                            if g == 0 or j == 0:
                                nc.vector.tensor_copy(dst, tpr)
                            else:
                                nc.scalar.copy(dst, tpr)
                            dsth = dsttile[D:P, :].rearrange(
                                "p (c e) -> p e c", e=8)[:, 4 * g:4 * (g + 1), :]
                            nc.scalar.copy(dsth, tpr)
                        else:
                            nc.scalar.copy(dst, tpr)


                # Ustack = [U1^T ; U2^T] [128, 1024] fp32
                ust = big.tile([P, T], fp32, tag="ust")
                for g in range(2):
                    up = usp.tile([P, 512], fp32, tag="us")
                    nc.tensor.matmul(up[:], ms_sb[:, j, :],
                                     qT[:, 512 * g:512 * (g + 1)])
                    nc.scalar.copy(ust[:, 512 * g:512 * (g + 1)], up[:])
                st[j] = {"kT2": kT2, "ust": ust}

            def back(j):
                s = st[j]
                kT2, ust = s["kT2"], s["ust"]
                c2 = big.tile([P, T], fp32, tag="c2")
                kt_r = big.tile([P, T], f32r, tag="kt_r")
                w_sb = big.tile([P, T], fp32, tag="w")
                v_sb = big.tile([P, T], f32r, tag="v")
                rden = big.tile([P, T], fp32, tag="rden")
                # chunked tail: scan (chained) -> kt_r -> W -> den ->
                # recip -> V per chunk, so the first chunk's chain and the
                # first output matmuls start long before the rest finishes.
                # bh0 uses quarter-granularity to shorten the pipeline fill.
                bounds = [0, 256, 512, 768, T] if j == 0 else [0, 512, T]
                kbounds = [0, 264, 512, 768, T] if j == 0 else [0, 512, T]
                for c in range(len(bounds) - 1):
                    lo, hi = bounds[c], bounds[c + 1]
                    slo = lo + 1 if c else 0
                    shi = min(hi + 1, T)
                    nc.vector.tensor_tensor_scan(
                        c2[:, slo:shi], kT2[:, slo:shi], kT2[:, slo:shi],
                        c2[:, slo - 1:slo] if c else 0.0,
                        mybir.AluOpType.add, mybir.AluOpType.bypass)
                    klo, khi = kbounds[c], kbounds[c + 1]
                    # f32r copy of kT2 for the output matmuls (Pool)
                    nc.gpsimd.tensor_copy(kt_r[:, klo:khi], kT2[:, klo:khi])
                    # W = Ustack * shifted(C2) (Pool)
                    if hi < T:
                        nc.gpsimd.tensor_mul(w_sb[:, lo:hi], ust[:, lo:hi],
                                             c2[:, lo + 1:hi + 1])
                    else:
                        nc.gpsimd.tensor_mul(w_sb[:, lo:T - 1],
                                             ust[:, lo:T - 1],
                                             c2[:, lo + 1:T])
                        nc.gpsimd.tensor_mul(w_sb[:, T - 1:T],
                                             ust[:, T - 1:T],
                                             c2[:, T - 1:T])
                    # den = on^T @ W (fp32); rden = 1/den; V = Ust*rden (f32r)
                    dp = denp.tile([P, hi - lo], fp32, tag="den",
                                   name=f"dp{c}")
                    nc.tensor.matmul(dp[:], on_sb[:], w_sb[:, lo:hi])
                    nc.vector.reciprocal(rden[:, lo:hi], dp[:])
                    nc.gpsimd.tensor_mul(v_sb[:, lo:hi], ust[:, lo:hi],
                                         rden[:, lo:hi])
                st[j] = {"kt_r": kt_r, "v": v_sb}

            def out(j, tiles=None):
                pr, jj = divmod(j, 2)
                s = st[j]
                kt_r, v_sb = s["kt_r"], s["v"]

                if tiles is not None:
                    order = tiles
                elif j == n_bh - 1:
                    order = range(NT - 1, -1, -1)
                else:
                    order = range(NT)
                for i in order:
                    wl = _live_width(i)
                    osb = outp.tile([P, _live_width(i)], fp32,
                                    tag=f"osb{i}", name=f"osb{i}")
                    # f32r matmul widths must be even; >=256 gets 1 cyc/row
                    def _f32r_pad(n):
                        ne = n + (n & 1)
                        return ne if ne >= 256 or 4 * ne < 258 else 258
                    n0 = min(wl, 512)
                    n0p = _f32r_pad(n0)
                    ow1 = owp.tile([P, 512], fp32, tag="ow")
                    nc.tensor.matmul(ow1[:, 0:n0p],
                                     v_sb[:, 128 * i:128 * (i + 1)],
                                     kt_r[:, 0:n0p])
                    if wl > 512:
                        n1p = _f32r_pad(wl - 512)
                        ow2 = owp.tile([P, 512], fp32, tag="ow")
                        nc.tensor.matmul(ow2[:, 0:n1p],
                                         v_sb[:, 128 * i:128 * (i + 1)],
                                         kt_r[:, 512:512 + n1p])

                    # drain psum -> osb: one DVE mask-mul per psum part
                    # covers its left cols AND the diagonal strip (mext view);
                    # full-ones ow1 parts for i>=4 are plain ACT copies
                    s0 = 128 * i
                    if s0 < 512:
                        e0 = min(wl, 512)
                        nc.vector.tensor_mul(osb[:, 0:e0], ow1[:, 0:e0],
                                             mext[:, 512 - s0:512 - s0 + e0])
                    else:
                        nc.scalar.copy(osb[:, 0:512], ow1[:, 0:512])
                    if wl > 512:
                        nc.vector.tensor_mul(
                            osb[:, 512:wl], ow2[:, 0:wl - 512],
                            mext[:, 1024 - s0:1024 - s0 + wl - 512])

                    nc.sync.dma_start(
                        out_d[j, 128 * i:128 * (i + 1), 0:wl], osb[:, 0:wl])
                if tiles is None or list(tiles)[-1] == NT - 1:
                    del st[j]

            # ---- software-pipelined emission ----
            # Per step: out(j) first so its ACT/DVE copies and SP DMAs sit at
            # the head of those FIFOs; then front(j+2) (PE T/U fill); then
            # back(j+1) whose den->recip->V tail overlaps the next step.
            def body():
                load_pair(0, split=True)
                act_warmup()
                load_consts()
                front(0)
                back(0)
                if n_bh > 1:
                    front(1)
                for j in range(n_bh):
                    out(j)
                    if j + 2 < n_bh:
                        front(j + 2)
                    if j + 1 < n_bh:
                        back(j + 1)
                pair_st.clear()
                st.clear()

            if repeat == 1:
                body()
            else:
                with tc.For_i(0, repeat, 1):
                    body()

    nc.compile()
    _BUILD_CACHE[(n_bh, repeat)] = nc
    return nc


def _host_prep(W1_q, W1_k, W2_q, W2_k, lambda_q1, lambda_k1, lambda_q2,
               lambda_k2):
    lam1 = np.exp(np.asarray(lambda_q1, np.float64).dot(
        np.asarray(lambda_k1, np.float64)))
    lam2 = np.exp(np.asarray(lambda_q2, np.float64).dot(
        np.asarray(lambda_k2, np.float64)))
    lam = np.float32(np.float32(lam1) - np.float32(lam2) + np.float32(LAMBDA_INIT))
    M1 = np.einsum("hde,hfe->hdf", W1_q.astype(np.float32),
                   W1_k.astype(np.float32)).astype(np.float32)
    M2 = np.einsum("hde,hfe->hdf", W2_q.astype(np.float32),
                   W2_k.astype(np.float32)).astype(np.float32)
    m_stack = np.concatenate([M1, M2], axis=2)          # [H, 64, 128]
    ones = np.zeros((128, 128), np.float32)
    ones[0:64, 0:64] = 1.0
    ones[64:128, 64:128] = np.float32(-1.0) / lam
    return m_stack, ones


def _make_in_maps(query_states, key_states, W1_q, W1_k, W2_q, W2_k,
                  lambda_q1, lambda_k1, lambda_q2, lambda_k2):
    q = np.ascontiguousarray(np.asarray(query_states, np.float32).reshape(BH, T, D))
    k = np.ascontiguousarray(np.asarray(key_states, np.float32).reshape(BH, T, D))
    m_stack, ones = _host_prep(W1_q, W1_k, W2_q, W2_k,
                               lambda_q1, lambda_k1, lambda_q2, lambda_k2)
    in_maps = []
    for c in range(NCORES):
        sl = slice(c * JPC, (c + 1) * JPC)
        hs = [bh % H for bh in range(c * JPC, (c + 1) * JPC)]
        in_maps.append({
            "q": np.ascontiguousarray(q[sl]),
            "k": np.ascontiguousarray(k[sl]),
            "ms": np.ascontiguousarray(m_stack[hs]),
            "on": ones,
        })
    return in_maps


def kernel(query_states, key_states, W1_q, W1_k, W2_q, W2_k,
           lambda_q1, lambda_k1, lambda_q2, lambda_k2):
    from concourse.bass_utils import run_bass_kernel_spmd

    in_maps = _make_in_maps(query_states, key_states, W1_q, W1_k, W2_q, W2_k,
                            lambda_q1, lambda_k1, lambda_q2, lambda_k2)
    nc = _build_module()
    res = run_bass_kernel_spmd(nc, in_maps, core_ids=list(range(NCORES)),
                               trace=False)
    out = np.empty((BH, T, T), np.float32)
    for c in range(NCORES):
        out[c * JPC:(c + 1) * JPC] = res.results[c]["out"]
    return out.reshape(B, H, T, T)

